# revision 10
# baseline (speedup 1.0000x reference)
"""Trainium2 Bass kernel for BayesianSTDPAdaptive (nn_BayesianSTDPAdaptive_9182640079056).

Computation (reference):
    iterations = 200, TB = 10, I = 1024, O = 512
    psp  = input_psp.reshape(200, 10, 1024)
    spk  = output_spikes.reshape(200, 10, 512)
    tos  = spk.sum(1)                       # (200, 512)
    corr = einsum('tbo,tbi->toi', spk, psp) # (200, 512, 1024)
    tsum = tos.sum(1)                       # (200,)
    scan over t:
        w += mu_w * (corr[t] * exp(-w) - tos[t][:, None])
        b += mu_b * ((exp(-b) * tos[t] - 1) * tsum[t])
    returns (w, b)

Strategy (8 cores = 4 o-shards x 2 i-shards):
    Each core owns a (128, 512) block of w (o rows 128*oi, i cols 512*ii) and
    computes corr for its block on the fly each scan step with a K=10 matmul
    (spk_t^T @ psp_t), packed 4-wide across PE row groups via tile_position.
    No cross-core communication at all; b is computed redundantly on every
    core (tiny) and harvested from the ii==0 cores.

    Sign trick: track a = -w.  Then per step:
        ew  = exp(a)                                  [ACT]
        t1n = (corr * -1) * ew                        [DVE scalar_tensor_tensor]
        a   = (t1n + tos_col) + a                     [DVE/Pool scalar_tensor_tensor]
    which avoids any separate negate / subtract instructions.
"""

import numpy as np

T = 2000
TB = 10
I_FULL = 1024
O_FULL = 512
IT = 200          # iterations
NTAU = 50         # IT / 4 row groups
OS = 128          # o rows per core
IS = 512          # i cols per core
W = 513           # state width per core (512 w cols + 1 b col)

# ---- tunables -------------------------------------------------------------
NCHUNK = 2                      # independent column chunks for pipelining
BT = 10                         # psp taus per DMA block
PSUM_BUFS = 4
# engine for the add-stt of each chunk: "pool" or "vector"
# NOTE: walrus rejects TensorScalarPtr on Pool, so "pool" is only usable for
# plain tensor_tensor ops.
ADD_ENG = ["vector"] * 8
MULT_ENG = ["vector"] * 8
# ---------------------------------------------------------------------------

_CACHE = {}


def _build_program(general_mu: bool):
    import concourse.bacc as bacc
    from concourse import mybir
    from concourse.tile import TileContext

    fp32 = mybir.dt.float32
    nc = bacc.Bacc()

    spk4_d = nc.dram_tensor("spk4", [128, NTAU * 128], fp32, kind="ExternalInput")
    psp4_d = nc.dram_tensor("psp4", [128, NTAU * 512], fp32, kind="ExternalInput")
    spkko_d = nc.dram_tensor("spkko", [128, 16 * 512], fp32, kind="ExternalInput")
    sel_d = nc.dram_tensor("sel", [128, 16 * 200], fp32, kind="ExternalInput")
    winit_d = nc.dram_tensor("winit", [128, W], fp32, kind="ExternalInput")
    if general_mu:
        mu_d = nc.dram_tensor("mu", [128, W], fp32, kind="ExternalInput")
    aout_d = nc.dram_tensor("aout", [128, W], fp32, kind="ExternalOutput")

    chunk_bounds = []
    step = IS // NCHUNK
    for ci in range(NCHUNK):
        chunk_bounds.append((ci * step, (ci + 1) * step))

    def eng(name):
        return {"pool": nc.gpsimd, "vector": nc.vector}[name]

    with TileContext(nc) as tc:
        with (
            tc.tile_pool(name="singles", bufs=1) as singles,
            tc.tile_pool(name="state", bufs=1) as state,
            tc.tile_pool(name="psp", bufs=2) as psp_pool,
            tc.tile_pool(name="work", bufs=2) as work,
            tc.tile_pool(name="psum", bufs=PSUM_BUFS, space="PSUM") as psum_pool,
            tc.tile_pool(name="prepsum", bufs=2, space="PSUM") as prepsum,
        ):
            # ---------------- load constants / inputs -----------------
            spk4 = singles.tile([128, NTAU * 128], fp32)
            nc.sync.dma_start(out=spk4, in_=spk4_d[:, :])
            spkko = singles.tile([128, 16 * 512], fp32)
            nc.sync.dma_start(out=spkko, in_=spkko_d[:, :])
            sel = singles.tile([128, 16 * 200], fp32)
            nc.sync.dma_start(out=sel, in_=sel_d[:, :])
            winit = singles.tile([128, W], fp32)
            nc.sync.dma_start(out=winit, in_=winit_d[:, :])
            if general_mu:
                mu_t = singles.tile([128, W], fp32)
                nc.sync.dma_start(out=mu_t, in_=mu_d[:, :])
            ones_t = singles.tile([128, 128], fp32)
            nc.vector.memset(ones_t, 1.0)

            # ---------------- precompute tos / tsum / c_all ------------
            # tos_sb[j][p, t] = sum_tb spikes[10t+tb, o] for o = o-chunk j
            # (host rotated spkko columns so j==0 is this core's own shard)
            tos_sb = []
            for j in range(4):
                pt = prepsum.tile([128, 200], fp32)
                for c in range(16):
                    nc.tensor.matmul(
                        pt,
                        spkko[:, c * 512 + 128 * j:c * 512 + 128 * (j + 1)],
                        sel[:, c * 200:(c + 1) * 200],
                        start=(c == 0),
                        stop=(c == 15),
                    )
                sb = singles.tile([128, 200], fp32, tag=f"tos{j}")
                nc.vector.tensor_copy(sb, pt)
                tos_sb.append(sb)
            tos = tos_sb[0]   # this core's o-shard

            # tsumb[p, t] = tsum[t] broadcast to all partitions
            pt = prepsum.tile([128, 200], fp32)
            for j in range(4):
                nc.tensor.matmul(pt, ones_t, tos_sb[j], start=(j == 0), stop=(j == 3))
            tsumb = singles.tile([128, 200], fp32)
            nc.vector.tensor_copy(tsumb, pt)

            # c_all[p, t] = tos[p, t] * tsum[t]
            c_all = singles.tile([128, 200], fp32)
            nc.vector.scalar_tensor_tensor(
                out=c_all, in0=tos, scalar=1.0, in1=tsumb,
                op0=mybir.AluOpType.mult, op1=mybir.AluOpType.mult)

            # ---------------- init state a = -winit --------------------
            a_c = []
            for ci in range(NCHUNK):
                c0, c1 = chunk_bounds[ci]
                at = state.tile([128, c1 - c0], fp32, tag=f"a{ci}")
                nc.vector.tensor_scalar_mul(at, winit[:, c0:c1], -1.0)
                a_c.append(at)
            a_b = state.tile([128, 1], fp32)
            nc.vector.tensor_scalar_mul(a_b, winit[:, 512:513], -1.0)

            # warm up the exp table during precompute
            dummy = work.tile([128, 1], fp32, tag="dummy")
            nc.scalar.activation(dummy, a_b, mybir.ActivationFunctionType.Exp)

            # ---------------- the scan ---------------------------------
            exp_f = mybir.ActivationFunctionType.Exp
            ALU = mybir.AluOpType
            pspblk = None
            for t in range(IT):
                tau, g = t // 4, t % 4
                if tau % BT == 0 and g == 0:
                    blk = tau // BT
                    pspblk = psp_pool.tile([128, BT * 512], fp32, tag="pspblk")
                    nc.sync.dma_start(
                        out=pspblk,
                        in_=psp4_d[:, blk * BT * 512:(blk + 1) * BT * 512])
                toff = (tau % BT) * 512

                # corr_t = spk_t^T @ psp_t  -> psum (128, 512)
                ps = psum_pool.tile([128, 512], fp32, tag="corr")
                nc.tensor.matmul(
                    ps,
                    spk4[32 * g:32 * g + TB, tau * 128:(tau + 1) * 128],
                    pspblk[32 * g:32 * g + TB, toff:toff + 512],
                    start=True, stop=True,
                    tile_position=(32 * g, 0),
                )

                tos_col = tos[:, t:t + 1]
                tsum_col = tsumb[:, t:t + 1]

                # b column ops (tiny)
                eb = work.tile([128, 1], fp32, tag="eb")
                nc.scalar.activation(eb, a_b, exp_f)
                t1b = work.tile([128, 1], fp32, tag="t1b")
                nc.vector.scalar_tensor_tensor(
                    out=t1b, in0=c_all[:, t:t + 1], scalar=-1.0, in1=eb,
                    op0=ALU.mult, op1=ALU.mult)
                if general_mu:
                    t2b = work.tile([128, 1], fp32, tag="t2b")
                    nc.vector.scalar_tensor_tensor(
                        out=t2b, in0=t1b, scalar=tsum_col, in1=mu_t[:, 512:513],
                        op0=ALU.add, op1=ALU.mult)
                    nc.vector.scalar_tensor_tensor(
                        out=a_b, in0=t2b, scalar=0.0, in1=a_b,
                        op0=ALU.bypass, op1=ALU.add)
                else:
                    nc.vector.scalar_tensor_tensor(
                        out=a_b, in0=t1b, scalar=tsum_col, in1=a_b,
                        op0=ALU.add, op1=ALU.add)

                for ci in range(NCHUNK):
                    c0, c1 = chunk_bounds[ci]
                    wdt = c1 - c0
                    ew = work.tile([128, wdt], fp32, tag=f"ew{ci}")
                    nc.scalar.activation(ew, a_c[ci], exp_f)
                    t1n = work.tile([128, wdt], fp32, tag=f"t1n{ci}")
                    eng(MULT_ENG[ci]).scalar_tensor_tensor(
                        out=t1n, in0=ps[:, c0:c1], scalar=-1.0, in1=ew,
                        op0=ALU.mult, op1=ALU.mult)
                    if general_mu:
                        t2n = work.tile([128, wdt], fp32, tag=f"t2n{ci}")
                        eng(ADD_ENG[ci]).scalar_tensor_tensor(
                            out=t2n, in0=t1n, scalar=tos_col, in1=mu_t[:, c0:c1],
                            op0=ALU.add, op1=ALU.mult)
                        eng(ADD_ENG[ci]).scalar_tensor_tensor(
                            out=a_c[ci], in0=t2n, scalar=0.0, in1=a_c[ci],
                            op0=ALU.bypass, op1=ALU.add)
                    else:
                        eng(ADD_ENG[ci]).scalar_tensor_tensor(
                            out=a_c[ci], in0=t1n, scalar=tos_col, in1=a_c[ci],
                            op0=ALU.add, op1=ALU.add)

            # ---------------- write out w = -a -------------------------
            outt = singles.tile([128, W], fp32, tag="outt")
            for ci in range(NCHUNK):
                c0, c1 = chunk_bounds[ci]
                nc.vector.tensor_scalar_mul(outt[:, c0:c1], a_c[ci], -1.0)
            nc.vector.tensor_scalar_mul(outt[:, 512:513], a_b, -1.0)
            nc.sync.dma_start(out=aout_d[:, :], in_=outt)

    return nc


def _prep_core_inputs(inputs, oi, ii):
    spk = np.ascontiguousarray(inputs["output_spikes"], dtype=np.float32)
    psp = np.ascontiguousarray(inputs["input_psp"], dtype=np.float32)
    weights = np.asarray(inputs["weights"], dtype=np.float32)
    biases = np.asarray(inputs["biases"], dtype=np.float32)

    o_sl = slice(128 * oi, 128 * (oi + 1))
    i_sl = slice(512 * ii, 512 * (ii + 1))

    # spk4[32g+tb, tau*128+ol] = spk[10*(4tau+g)+tb, 128oi+ol]
    sk = spk[:, o_sl].reshape(NTAU, 4, TB, 128).transpose(1, 2, 0, 3)
    arr = np.zeros((4, 32, NTAU, 128), np.float32)
    arr[:, :TB] = sk
    spk4 = arr.reshape(128, NTAU * 128)

    pp = psp[:, i_sl].reshape(NTAU, 4, TB, 512).transpose(1, 2, 0, 3)
    arr = np.zeros((4, 32, NTAU, 512), np.float32)
    arr[:, :TB] = pp
    psp4 = arr.reshape(128, NTAU * 512)

    # spkko[p, c*512+o] = spk_padded[128c+p, o] with o-chunks rolled so own
    # shard is first; spikes zero-padded from 2000 to 2048 rows
    col_order = np.concatenate(
        [np.arange(128 * j, 128 * (j + 1)) for j in np.roll(np.arange(4), -oi)])
    spk_pad = np.zeros((2048, 512), np.float32)
    spk_pad[:T] = spk[:, col_order]
    spkko = np.ascontiguousarray(
        spk_pad.reshape(16, 128, 512).transpose(1, 0, 2).reshape(128, 16 * 512))

    winit = np.zeros((128, W), np.float32)
    winit[:, :512] = weights[o_sl, i_sl]
    winit[:, 512] = biases[o_sl]

    return {"spk4": spk4, "psp4": psp4, "spkko": spkko, "winit": winit}


def _sel_mask():
    tt = np.arange(2048) // TB
    sel2 = (tt[:, None] == np.arange(IT)[None, :]).astype(np.float32)
    sel2[T:] = 0.0
    return np.ascontiguousarray(
        sel2.reshape(16, 128, IT).transpose(1, 0, 2).reshape(128, 16 * IT))


def kernel(input_psp, output_spikes, weights, biases, mu_weights, mu_bias,
           _trace=False, _trace_kwargs=None):
    from concourse import bass_utils

    general_mu = not (np.allclose(np.asarray(mu_weights), 1.0)
                      and np.allclose(np.asarray(mu_bias), 1.0))
    key = ("prog", general_mu)
    if key not in _CACHE:
        nc = _build_program(general_mu)
        if not nc.is_finalized():
            nc.finalize()   # runs Bacc passes (reg alloc, matmul wait splitting)
        _CACHE[key] = nc
    nc = _CACHE[key]

    inputs = {
        "input_psp": input_psp, "output_spikes": output_spikes,
        "weights": weights, "biases": biases,
    }
    sel = _sel_mask()
    in_maps = []
    for core in range(8):
        oi, ii = core // 2, core % 2
        m = _prep_core_inputs(inputs, oi, ii)
        m["sel"] = sel
        if general_mu:
            mu = np.zeros((128, W), np.float32)
            mu[:, :512] = np.asarray(mu_weights, np.float32)[
                128 * oi:128 * (oi + 1), 512 * ii:512 * (ii + 1)]
            mu[:, 512] = np.asarray(mu_bias, np.float32)[128 * oi:128 * (oi + 1)]
            m["mu"] = mu
        in_maps.append(m)

    res = bass_utils.run_bass_kernel_spmd(
        nc, in_maps, core_ids=list(range(8)),
        trace=_trace, **(_trace_kwargs or {}))

    w = np.zeros((O_FULL, I_FULL), np.float32)
    b = np.zeros((O_FULL,), np.float32)
    for core in range(8):
        oi, ii = core // 2, core % 2
        aout = res.results[core]["aout"]
        w[128 * oi:128 * (oi + 1), 512 * ii:512 * (ii + 1)] = aout[:, :512]
        if ii == 0:
            b[128 * oi:128 * (oi + 1)] = aout[:, 512]

    kernel._last_result = res
    return (w, b)
